# revision 7
# baseline (speedup 1.0000x reference)
"""CenterLoss Trainium2 kernel (label-sorted data-parallel over 8 NeuronCores).

loss = sum(clip(distmat * onehot(labels), 1e-12, 1e12)) / B,
distmat[i,c] = ||x_i - centers_c||^2. Only the (i, labels_i) entries survive
the mask; the B*(C-1) masked entries contribute exactly 1e-12 each (added
analytically on host). d_i ~ 4096 so the clip never binds and

  sum_i d_i = sum_i ||x_i||^2 + sum_c n_c ||c_c||^2 - 2 sum_c <s_c, c_c>

with s = onehot(labels)^T @ x and n = histogram(labels).

Sharding: samples are SORTED BY LABEL on host (a pure permutation — sum_i is
permutation-invariant) and split into 8 equal 1024-sample shards. Each
shard's labels then span < 128 consecutive classes (seed-0 spans are 90-99;
uniform labels give ~94 +- 7), so each core loads only a 128-row window of
centers (1.0 MB) instead of all 751 rows (6.15 MB). Per-core HBM traffic is
9.4 MB, within ~12% of the 8.4 MB cost-model floor for streaming the fp32 x
shard at the simulator's serialized 360 B/ns DMA bandwidth.

The class window also collapses the one-hot matmul to a single 128-class
tile: s is [128, 2048], computed on the PE in fp32r (TF32-style; 1 cycle/row
at >=256-wide outputs, measured 1.4e-4 rel vs fp32 on HW) directly from the
DMA-loaded x tiles bitcast to fp32r — no fp8/bf16 conversion pass on the
vector engine. ||x||^2 and ||c||^2 are exact f32 on the scalar engine;
-2<s,c> is fused into the PSUM drain via DVE scalar_tensor_tensor (GPSIMD
cannot access PSUM). Two PSUM accumulation groups (sample tiles 0-3 / 4-7,
8 banks total) let half the drains run mid-flight instead of on the tail.

Per-core output is a [128, 20] block of raw partial columns:
  cols 0..7   sum_d x[i,d]^2 per sample-tile t (i = t*128 + partition)
  cols 8..15  -2*<s_c, c_c> per (psum group g, 512-col chunk n): col 8+4g+n
  col  16     cn2: ||c_c||^2 for local class c = partition
Host combine (f64): sum cols 0..15 over cores + <histogram, cn2> with each
class read from the one core whose window owns it + B*(C-1)*1e-12, / B.
"""

from contextlib import ExitStack

import numpy as np

import concourse.bacc as bacc
import concourse.tile as tile
from concourse import mybir
from concourse.bass_utils import run_bass_kernel_spmd

N_CORES = 8
B = 8192
D = 2048
C = 751
BS = B // N_CORES  # samples per core
P = 128
NT = BS // P       # sample tiles per core
CW = 128           # centers window rows per core (max label span per shard)
NCH = D // 512     # feature chunks (one PSUM bank each)
NG = 2             # psum accumulation groups (tiles 0..3 and 4..7)
TPG = NT // NG     # tiles per group
OUTW = 20
F32 = mybir.dt.float32
F32R = mybir.dt.float32r
CLIP_LO = 1e-12

_NC = None


def build_nc():
    nc = bacc.Bacc("TRN2", target_bir_lowering=False)
    x = nc.dram_tensor("x", [BS, D], F32, kind="ExternalInput")
    labels = nc.dram_tensor("labels", [P, NT], mybir.dt.int32, kind="ExternalInput")
    cwin = nc.dram_tensor("cwin", [CW, D], F32, kind="ExternalInput")
    out = nc.dram_tensor("partial", [P, OUTW], F32, kind="ExternalOutput")

    # x_r[p, t, :] = x[t*128 + p, :]
    x_r = x[:].rearrange("(t p) d -> p t d", p=P)

    with tile.TileContext(nc) as tc, ExitStack() as ctx:
        xp = ctx.enter_context(tc.tile_pool(name="xp", bufs=3))
        sqp = ctx.enter_context(tc.tile_pool(name="sqp", bufs=2))
        perm = ctx.enter_context(tc.tile_pool(name="perm", bufs=1))
        psp = ctx.enter_context(tc.tile_pool(name="psp", bufs=1, space="PSUM"))

        # labels ride the ACT HWDGE ring so the x loads' SP ring is unblocked
        lab = perm.tile([P, NT], mybir.dt.int32)
        nc.scalar.dma_start(out=lab[:], in_=labels[:])
        lab_f = perm.tile([P, NT], F32)
        nc.vector.tensor_copy(out=lab_f[:], in_=lab[:])

        iota_i = perm.tile([P, CW], mybir.dt.int32)
        nc.gpsimd.iota(iota_i[:], pattern=[[1, CW]], base=0, channel_multiplier=0)
        iota_f = perm.tile([P, CW], F32)
        nc.vector.tensor_copy(out=iota_f[:], in_=iota_i[:])

        out_sb = perm.tile([P, OUTW], F32)
        nc.vector.memset(out_sb[:], 0.0)

        # one-hot lhsT tiles, produced directly as fp32r (0.0/1.0 are exact)
        oh = perm.tile([P, NT, CW], F32R)
        for t in range(NT):
            nc.vector.tensor_scalar(
                out=oh[:, t, :], in0=iota_f[:], scalar1=lab_f[:, t : t + 1],
                scalar2=None, op0=mybir.AluOpType.is_equal,
            )

        ct = perm.tile([P, D], F32)
        ps = []
        for g in range(NG):
            row = []
            for n in range(NCH):
                ps_gn = psp.tile([P, 512], F32, tag=f"ps{g}_{n}")
                row.append(ps_gn)
            ps.append(row)

        def drain(g, n):
            stt_o = sqp.tile([P, 512], F32, tag="stt_o")
            nc.vector.scalar_tensor_tensor(
                out=stt_o[:], in0=ps[g][n][:], scalar=-2.0,
                in1=ct[:, n * 512 : (n + 1) * 512],
                op0=mybir.AluOpType.mult, op1=mybir.AluOpType.mult,
                accum_out=out_sb[:, 8 + NCH * g + n : 9 + NCH * g + n],
            )

        for t in range(NT):
            g = t // TPG
            last = t == NT - 1
            xt = xp.tile([P, D], F32R, tag="xt")
            if not last:
                nc.sync.dma_start(out=xt[:], in_=x_r[:, t, :].bitcast(F32R))
                for n in range(NCH):
                    nc.tensor.matmul(
                        out=ps[g][n][:], lhsT=oh[:, t, :],
                        rhs=xt[:, n * 512 : (n + 1) * 512],
                        start=(t % TPG == 0), stop=False,
                    )
            else:
                # Final tile streams in 512-column quarters: each PSUM bank's
                # closing matmul, drain, and quarter-square fire as its
                # quarter lands, so only one of each trails the last transfer.
                # Quarter-square accums go to cols 7,17,18,19 (host sums all).
                for n in range(NCH):
                    sl = slice(n * 512, (n + 1) * 512)
                    nc.sync.dma_start(out=xt[:, sl], in_=x_r[:, t, sl].bitcast(F32R))
                    nc.tensor.matmul(
                        out=ps[g][n][:], lhsT=oh[:, t, :], rhs=xt[:, sl],
                        start=False, stop=True,
                    )
                    drain(g, n)
                    qcol = t if n == 0 else 16 + n
                    sqq = sqp.tile([P, 512], F32, tag="stt_o")
                    nc.scalar.activation(
                        out=sqq[:], in_=xt[:, sl].bitcast(F32),
                        func=mybir.ActivationFunctionType.Square,
                        accum_out=out_sb[:, qcol : qcol + 1],
                    )
            if not last:
                sq = sqp.tile([P, D], F32, tag="sq")
                nc.scalar.activation(
                    out=sq[:], in_=xt[:].bitcast(F32),
                    func=mybir.ActivationFunctionType.Square,
                    accum_out=out_sb[:, t : t + 1],
                )
            if t == 1:
                # centers window: off the critical tail, before drain A needs it
                nc.sync.dma_start(out=ct[:], in_=cwin[:])
                sqc = sqp.tile([P, D], F32, tag="sq")
                nc.scalar.activation(
                    out=sqc[:], in_=ct[:],
                    func=mybir.ActivationFunctionType.Square,
                    accum_out=out_sb[:, 16:17],
                )
            if t == TPG - 1:
                for n in range(NCH):
                    drain(0, n)

        nc.sync.dma_start(out=out[:], in_=out_sb[:])
    nc.compile()
    return nc


def _shard(x, labels, centers):
    """Sort samples by label, split into 8 equal shards, slice the centers
    window each shard's labels fall in. Returns (in_maps, lo_list)."""
    order = np.argsort(labels, kind="stable")
    xs = x[order]
    ls = labels[order]
    in_maps, los = [], []
    for k in range(N_CORES):
        chunk = ls[k * BS : (k + 1) * BS]
        lo = int(chunk[0])
        assert int(chunk[-1]) - lo < CW, (
            f"shard {k} label span {int(chunk[-1]) - lo + 1} exceeds window {CW}"
        )
        rows = min(CW, C - lo)
        cw = np.zeros((CW, D), dtype=np.float32)
        cw[:rows] = centers[lo : lo + rows]
        # lab[p, t] = local label of sample t*P + p, matching the x tile layout
        lab = np.ascontiguousarray((chunk - lo).astype(np.int32).reshape(NT, P).T)
        in_maps.append({
            "x": np.ascontiguousarray(xs[k * BS : (k + 1) * BS]),
            "labels": lab,
            "cwin": cw,
        })
        los.append(lo)
    return in_maps, los


def make_in_maps(x, labels, centers):
    return _shard(x, labels, centers)[0]


def combine_partials(partials, los, labels):
    total = 0.0
    for p in partials:
        total += float(np.sum(p[:, :16].astype(np.float64)))
        total += float(np.sum(p[:, 17:].astype(np.float64)))
    # n_c * ||c_c||^2: host histogram x device cn2, each class read from the
    # one core whose window owns it (largest k with lo_k <= c)
    hist = np.bincount(np.asarray(labels).astype(np.int64), minlength=C)
    los = np.asarray(los)
    for c in np.nonzero(hist)[0]:
        k = int(np.searchsorted(los, c, side="right")) - 1
        i = int(c) - int(los[k])
        assert 0 <= i < CW
        total += float(hist[c]) * float(partials[k][i, 16])
    total += float(B) * float(C - 1) * CLIP_LO
    return np.array(total / B, dtype=np.float32)


def kernel(**inputs) -> np.ndarray:
    global _NC
    x = np.ascontiguousarray(np.asarray(inputs["x"], dtype=np.float32))
    labels = np.asarray(inputs["labels"]).astype(np.int64)
    centers = np.ascontiguousarray(np.asarray(inputs["centers"], dtype=np.float32))
    assert x.shape == (B, D) and labels.shape == (B,) and centers.shape == (C, D)

    if _NC is None:
        _NC = build_nc()
    in_maps, los = _shard(x, labels, centers)
    res = run_bass_kernel_spmd(_NC, in_maps, core_ids=list(range(N_CORES)))
    return combine_partials([r["partial"] for r in res.results], los, labels)
